# revision 23
# baseline (speedup 1.0000x reference)
"""Trainium2 Bass kernel for nn_Attention_57543971831928.

Dense pre-LN self-attention block:
  t = x.reshape(B,C,N).T ; t_norm = LN(t)
  qkv = t_norm @ W1.T + b1 ; attention (6 heads, d=64) ; o = att_out @ W2.T + b2
  out = (o + t_norm).T.reshape(B,C,H,W)

Sharding: data-parallel over batch B=8, one batch element per NeuronCore,
no collectives.  Inside each core everything is computed in the
"transposed" [c, n] / [j, n] layout so every matmul contraction sits on
the partition axis:

  - LayerNorm stats need per-n reductions over c, so x is PE-transposed
    to [n, c] tiles, normalized there (bn_stats/bn_aggr), and the bf16
    result transposed back to [c, n].
  - q^T/k^T [j, n] = W1^T-stationary matmuls; V [n, j] uses t_norm^T
    blocks as the stationary operand (saves transposing V later).
  - Scores are computed transposed, S^T[n_j, n_i] = k_h^T.T @ q_h^T,
    with two heads packed into the 128x128 PE array as 64-row tiles
    (K=d=64 each).  exp() runs on ScalarE straight out of PSUM for both
    heads in one instruction (softmax denominator is deferred).
  - PV uses E^T blocks as the stationary operand and V_h augmented with
    a ones column, so O'[n_i, 0:64] and the softmax denominator
    r[n_i] = O'[n_i, 64] come out of the same accumulation.  A
    reciprocal + scalar-mul normalizes afterwards.

The softmax exp is the roofline: B*h*N^2/8 = 31.85M elements/core
through ScalarE at 128 lanes * 1.2 GHz ~= 210 us; everything else is
overlapped against it.
"""

import sys

sys.path.insert(0, "/opt/trn_rl_repo")

import numpy as np
import orjson

import concourse.bass as bass
import concourse.mybir as mybir
import concourse.tile as tile
from concourse.masks import make_identity

# ---------------------------------------------------------------------------
# Workaround: the walrus build in this container only accepts a single
# sync-wait per instruction; Tile emits instructions waiting on several
# semaphores.  Split extra waits onto preceding same-engine NoOps at BIR
# serialization time.
# ---------------------------------------------------------------------------
_SYNC_WAIT_LIMIT = 1


def _fix_inst_list(insts):
    out = []
    for inst in insts:
        si = inst.get("sync_info")
        ow = (si or {}).get("on_wait") or []
        if si is not None and len(ow) > _SYNC_WAIT_LIMIT:
            keep = ow[-_SYNC_WAIT_LIMIT:]
            extras = ow[:-_SYNC_WAIT_LIMIT]
            for i, w in enumerate(extras):
                out.append(
                    {
                        "debug": inst.get("debug", 0),
                        "engine": inst["engine"],
                        "ins": [],
                        "outs": [],
                        "name": f"{inst['name']}.w{i}",
                        "opcode": "NoOp",
                        "sync_info": {"on_update": [], "on_wait": [w]},
                    }
                )
            si["on_wait"] = keep
        out.append(inst)
    return out


def _walk_fix(obj):
    if isinstance(obj, dict):
        for k, v in obj.items():
            if k == "instructions" and isinstance(v, list):
                obj[k] = _fix_inst_list(v)
                for inst in obj[k]:
                    _walk_fix(inst)
            else:
                _walk_fix(v)
    elif isinstance(obj, list):
        for v in obj:
            _walk_fix(v)


if not getattr(bass.Bass, "_ant_wait_split_patched", False):
    _orig_to_json_bytes = bass.Bass.to_json_bytes

    def _patched_to_json_bytes(self):
        m = orjson.loads(_orig_to_json_bytes(self))
        _walk_fix(m)
        return orjson.dumps(m)

    bass.Bass.to_json_bytes = _patched_to_json_bytes
    bass.Bass._ant_wait_split_patched = True

# ---------------------------------------------------------------------------
# Problem constants (hardcoded per task instructions)
# ---------------------------------------------------------------------------
B = 8
C = 384
H = W = 48
N = H * W          # 2304
NH = 6             # heads
D = C // NH        # 64
J3 = 3 * C         # 1152
P = 128
CT = C // P        # 3 c-tiles
NT = N // P        # 18 n-tiles
LN_EPS = 1e-5

F32 = mybir.dt.float32
BF16 = mybir.dt.bfloat16



def build_nc(reps: int = 1) -> bass.Bass:
    nc = bass.Bass()
    x_ext = nc.dram_tensor("x", [C, N], F32, kind="ExternalInput")
    w1_ext = nc.dram_tensor("W1", [J3, C], F32, kind="ExternalInput")
    b1_ext = nc.dram_tensor("b1", [J3], F32, kind="ExternalInput")
    w2_ext = nc.dram_tensor("W2", [C, C], F32, kind="ExternalInput")
    b2_ext = nc.dram_tensor("b2", [C], F32, kind="ExternalInput")
    out_ext = nc.dram_tensor("out", [C, N], F32, kind="ExternalOutput")

    with tile.TileContext(nc) as tc:
        for _ in range(reps):
            _build_body(nc, tc, x_ext, w1_ext, b1_ext, w2_ext, b2_ext, out_ext)
    return nc


def _build_body(nc, tc, x_ext, w1_ext, b1_ext, w2_ext, b2_ext, out_ext):
    from contextlib import ExitStack

    with ExitStack() as ctx:
        singles = ctx.enter_context(tc.tile_pool(name="singles", bufs=1))

        # ---- constants -----------------------------------------------------
        ident_f32 = singles.tile([P, P], F32)
        make_identity(nc, ident_f32)
        ident_bf = singles.tile([P, P], BF16)
        make_identity(nc, ident_bf)
        eps_sb = singles.tile([P, 1], F32)
        nc.vector.memset(eps_sb, LN_EPS)

        # b1 laid out partition-major per j-tile: b1_sb[p, jt] = b1[jt*128+p]
        b1_ap = b1_ext[:]
        b2_ap = b2_ext[:]
        b1_sb = singles.tile([P, J3 // P], F32)
        nc.sync.dma_start(
            out=b1_sb,
            in_=bass.AP(tensor=b1_ap.tensor, offset=b1_ap.offset,
                        ap=[[1, P], [P, J3 // P]]),
        )
        b2_sb = singles.tile([P, C // P], F32)
        nc.sync.dma_start(
            out=b2_sb,
            in_=bass.AP(tensor=b2_ap.tensor, offset=b2_ap.offset,
                        ap=[[1, P], [P, C // P]]),
        )
        # b1 slice for V, single row (broadcast via K=1 matmul later)
        b1v_f32 = singles.tile([1, C], F32)
        nc.sync.dma_start(
            out=b1v_f32,
            in_=bass.AP(tensor=b1_ap.tensor, offset=b1_ap.offset + 2 * C,
                        ap=[[1, 1], [1, C]]),
        )
        b1v_sb = singles.tile([1, C], BF16)
        nc.vector.tensor_copy(b1v_sb, b1v_f32)

        # ---- W1^T / W2^T (bf16, [c, j] layout) ----------------------------
        w1t_sb = [singles.tile([P, J3], BF16, name=f"w1t{i}") for i in range(CT)]
        w2t_sb = [singles.tile([P, C], BF16, name=f"w2t{i}") for i in range(CT)]

        with (
            tc.tile_pool(name="wrows", bufs=3) as wrows,
            tc.tile_pool(name="wpsum", bufs=4, space="PSUM") as wpsum,
        ):
            for jt in range(J3 // P):
                wr = wrows.tile([P, C], F32, tag="wrow")
                nc.sync.dma_start(out=wr, in_=w1_ext[jt * P:(jt + 1) * P, :])
                for ct in range(CT):
                    ps = wpsum.tile([P, P], F32, tag="wT")
                    nc.tensor.transpose(ps, wr[:, ct * P:(ct + 1) * P], ident_f32)
                    nc.any.tensor_copy(w1t_sb[ct][:, jt * P:(jt + 1) * P], ps)
            for rt in range(CT):
                wr = wrows.tile([P, C], F32, tag="wrow")
                nc.sync.dma_start(out=wr, in_=w2_ext[rt * P:(rt + 1) * P, :])
                for ct in range(CT):
                    ps = wpsum.tile([P, P], F32, tag="wT")
                    nc.tensor.transpose(ps, wr[:, ct * P:(ct + 1) * P], ident_f32)
                    nc.any.tensor_copy(w2t_sb[ct][:, rt * P:(rt + 1) * P], ps)

        # ---- persistent activations ---------------------------------------
        tn_cn = [singles.tile([P, N], BF16, name=f"tn_cn{i}") for i in range(CT)]
        qkT = [singles.tile([P, N], BF16, name=f"qkT{i}") for i in range(2 * C // P)]
        v_sb = [singles.tile([P, NH, D + 1], BF16, name=f"v{i}") for i in range(NT)]
        o_nc = [singles.tile([P, NH, D], BF16, name=f"o_nc{i}") for i in range(NT)]

        # ---- LayerNorm -----------------------------------------------------
        with (
            tc.tile_pool(name="xin", bufs=1) as xin,
            tc.tile_pool(name="ln", bufs=4) as ln,
            tc.tile_pool(name="lnps", bufs=5, space="PSUM") as lnps,
            tc.tile_pool(name="tn_nc_pool", bufs=4) as tn_nc_pool,
            tc.tile_pool(name="tps", bufs=3, space="PSUM") as tps,
        ):
            x_sb = [xin.tile([P, N], F32, name=f"x_sb{i}") for i in range(CT)]
            dma_engines = [nc.sync, nc.scalar, nc.gpsimd]
            for ct in range(CT):
                dma_engines[ct].dma_start(out=x_sb[ct],
                                          in_=x_ext[ct * P:(ct + 1) * P, :])

            for nt in range(NT):
                pt = lnps.tile([P, C], F32, tag="xt")      # t tile [n, c]
                for ct in range(CT):
                    nc.tensor.transpose(
                        pt[:, ct * P:(ct + 1) * P],
                        x_sb[ct][:, nt * P:(nt + 1) * P],
                        ident_f32,
                    )
                stats = ln.tile([P, nc.vector.BN_STATS_DIM], F32, tag="stats")
                nc.vector.bn_stats(out=stats, in_=pt)
                mv = ln.tile([P, nc.vector.BN_AGGR_DIM], F32, tag="mv")
                nc.vector.bn_aggr(out=mv, in_=stats)
                rstd = ln.tile([P, 1], F32, tag="rstd")
                nc.scalar.activation(
                    out=rstd, in_=mv[:, 1:2],
                    func=mybir.ActivationFunctionType.Sqrt,
                    bias=eps_sb, scale=1.0, alpha=0.0,
                )
                nc.vector.reciprocal(out=rstd, in_=rstd)
                tn = tn_nc_pool.tile([P, C], BF16, tag="tn_nc")
                nc.vector.tensor_scalar(
                    out=tn, in0=pt,
                    scalar1=mv[:, 0:1], scalar2=rstd,
                    op0=mybir.AluOpType.subtract, op1=mybir.AluOpType.mult,
                )
                # transpose t_norm back to [c, n]
                for ct in range(CT):
                    pc = tps.tile([P, P], BF16, tag="tnT")
                    nc.tensor.transpose(pc, tn[:, ct * P:(ct + 1) * P], ident_bf)
                    nc.scalar.copy(tn_cn[ct][:, nt * P:(nt + 1) * P], pc)

        # ---- QKV -----------------------------------------------------------
        N_SUBS = [(s, min(512, N - s)) for s in range(0, N, 512)]
        with tc.tile_pool(name="qkps", bufs=4, space="PSUM") as qkps:
            for jt in range(2 * C // P):  # q^T and k^T row-tiles [j, n]
                for s0, sl in N_SUBS:
                    ps = qkps.tile([P, 512], F32, tag="qk")
                    for ct in range(CT):
                        nc.tensor.matmul(
                            ps[:, :sl],
                            w1t_sb[ct][:, jt * P:(jt + 1) * P],
                            tn_cn[ct][:, s0:s0 + sl],
                            start=(ct == 0), stop=(ct == CT - 1),
                        )
                    nc.scalar.activation(
                        out=qkT[jt][:, s0:s0 + sl], in_=ps[:, :sl],
                        func=mybir.ActivationFunctionType.Identity,
                        bias=b1_sb[:, jt:jt + 1], scale=1.0,
                    )

            ones_row = singles.tile([1, P], BF16, name="ones_row")
            nc.vector.memset(ones_row, 1.0)
            for nt in range(NT):  # V in [n, j] layout, with ones column
                ps = qkps.tile([P, C], F32, tag="v")
                for ct in range(CT):
                    nc.tensor.matmul(
                        ps,
                        tn_cn[ct][:, nt * P:(nt + 1) * P],
                        w1t_sb[ct][:, 2 * C:3 * C],
                        start=(ct == 0), stop=False,
                    )
                # + b1v broadcast to every row via a K=1 ones-row matmul
                nc.tensor.matmul(ps, ones_row, b1v_sb[0:1, :],
                                 start=False, stop=True)
                nc.vector.memset(v_sb[nt][:, :, D:D + 1], 1.0)
                nc.vector.tensor_copy(
                    v_sb[nt].rearrange("p h d -> p (h d)")[:, : NH * (D + 1)]
                    .rearrange("p (h d) -> p h d", h=NH)[:, :, 0:D],
                    ps.rearrange("p (h d) -> p h d", h=NH),
                )

        # ---- attention + projection, n_i-chunk outer ----------------------
        # Per 512-wide n_i chunk: all 3 head pairs run S^T -> exp -> PV,
        # normalize into o_cn[:, chunk]; then the output projection +
        # residual for that chunk issues immediately (overlaps the next
        # chunk's attention on PE/DVE while ScalarE stays exp-bound).
        o_cn = [singles.tile([P, N], BF16, name=f"o_cn{i}") for i in range(CT)]
        CHUNKS = [(s, min(512, N - s)) for s in range(0, N, 512)]
        with (
            tc.tile_pool(name="et", bufs=3) as etp,
            tc.tile_pool(name="sps", bufs=2, space="PSUM") as sps,
            tc.tile_pool(name="ops", bufs=1, space="PSUM") as ops,
            tc.tile_pool(name="rbps", bufs=1, space="PSUM") as rbps,
            tc.tile_pool(name="pps", bufs=1, space="PSUM") as pps,
            tc.tile_pool(name="nrm", bufs=4) as nrm,
            tc.tile_pool(name="outp", bufs=3) as outp,
        ):
            ones_bf = singles.tile([1, D], BF16, name="ones_bf")
            nc.vector.memset(ones_bf, 1.0)
            for c0, cl in CHUNKS:
                for hp in range(NH // 2):  # head pairs (2hp, 2hp+1)
                    po = [ops.tile([P, 512], F32, tag=f"O{i}", name=f"po{i}")
                          for i in range(2)]
                    for njt in range(NT):
                        ps_s = sps.tile([P, 2, 512], F32, tag="S")
                        for h2 in range(2):
                            nc.tensor.matmul(
                                ps_s[:, h2, 0:cl],
                                qkT[NH // 2 + hp][h2 * D:(h2 + 1) * D,
                                                  njt * P:(njt + 1) * P],
                                qkT[hp][h2 * D:(h2 + 1) * D, c0:c0 + cl],
                                start=True, stop=True,
                            )
                        et = etp.tile([P, 2, 512], BF16, tag="ET")
                        nc.scalar.activation(
                            out=et[:, :, 0:cl], in_=ps_s[:, :, 0:cl],
                            func=mybir.ActivationFunctionType.Exp,
                            scale=0.125,
                        )
                        for h2 in range(2):
                            nc.tensor.matmul(
                                po[h2][0:D + 1, 0:cl],
                                v_sb[njt][:, 2 * hp + h2, :],
                                et[:, h2, 0:cl],
                                start=(njt == 0), stop=(njt == NT - 1),
                            )
                    # Stage O' out of PSUM immediately (frees po for the next
                    # pair), then normalize off the critical path: recip row ->
                    # bf16 -> K=1 ones-matmul broadcast across 64 psum
                    # partitions -> elementwise multiply into o_cn.
                    for h2 in range(2):
                        ou = nrm.tile([P, 512], F32, tag="ou")
                        nc.vector.tensor_copy(ou[0:D + 1, 0:cl],
                                              po[h2][0:D + 1, 0:cl])
                        rrow = nrm.tile([1, 512], F32, tag="rrow")
                        nc.vector.reciprocal(out=rrow[:, 0:cl],
                                             in_=ou[D:D + 1, 0:cl])
                        rrow_bf = nrm.tile([1, 512], BF16, tag="rrow_bf")
                        nc.vector.tensor_copy(rrow_bf[:, 0:cl], rrow[:, 0:cl])
                        rbp = rbps.tile([P, 512], F32, tag="rb")
                        nc.tensor.matmul(
                            rbp[h2 * D:(h2 + 1) * D, 0:cl],
                            ones_bf,
                            rrow_bf[:, 0:cl],
                            start=True, stop=True,
                        )
                        nc.vector.tensor_tensor(
                            o_cn[hp][h2 * D:(h2 + 1) * D, c0:c0 + cl],
                            ou[0:D, 0:cl],
                            rbp[h2 * D:(h2 + 1) * D, 0:cl],
                            mybir.AluOpType.mult,
                        )
                # output projection + residual for this chunk
                for rt in range(CT):
                    ps = pps.tile([P, 512], F32, tag="proj")
                    for ct in range(CT):
                        nc.tensor.matmul(
                            ps[:, :cl],
                            w2t_sb[ct][:, rt * P:(rt + 1) * P],
                            o_cn[ct][:, c0:c0 + cl],
                            start=(ct == 0), stop=(ct == CT - 1),
                        )
                    out_sb = outp.tile([P, 512], F32, tag="out")
                    nc.vector.scalar_tensor_tensor(
                        out=out_sb[:, :cl],
                        in0=ps[:, :cl],
                        scalar=b2_sb[:, rt:rt + 1],
                        in1=tn_cn[rt][:, c0:c0 + cl],
                        op0=mybir.AluOpType.add,
                        op1=mybir.AluOpType.add,
                    )
                    nc.sync.dma_start(
                        out=out_ext[rt * P:(rt + 1) * P, c0:c0 + cl],
                        in_=out_sb[:, :cl])


# ---------------------------------------------------------------------------
# host-side entry point
# ---------------------------------------------------------------------------
_NC_CACHE = {}


def _get_nc(reps: int = 1):
    if reps not in _NC_CACHE:
        _NC_CACHE[reps] = build_nc(reps)
    return _NC_CACHE[reps]


def kernel(x, W1, b1, W2, b2):
    from concourse.bass_utils import run_bass_kernel_spmd

    nc = _get_nc()
    x = np.ascontiguousarray(x, dtype=np.float32)
    in_maps = [
        {
            "x": x[b].reshape(C, N),
            "W1": np.ascontiguousarray(W1, dtype=np.float32),
            "b1": np.ascontiguousarray(b1, dtype=np.float32),
            "W2": np.ascontiguousarray(W2, dtype=np.float32),
            "b2": np.ascontiguousarray(b2, dtype=np.float32),
        }
        for b in range(B)
    ]
    res = run_bass_kernel_spmd(nc, in_maps, core_ids=list(range(B)))
    out = np.stack([res.results[b]["out"] for b in range(B)], axis=0)
    return out.reshape(B, C, H, W).astype(np.float32)
